# revision 46
# baseline (speedup 1.0000x reference)
"""Trainium2 Bass kernel for nn_CustomCLIP (CLIP + batched Sinkhorn OT head).

Contract: kernel(**inputs) takes the FULL inputs of reference.setup_inputs()
and returns the FULL [32, 1000] output. Internally shards the image batch
b=32 across 8 NeuronCores (4 per core); text features are replicated.

Math notes:
  The reference runs Sinkhorn (eps=0.1) to convergence; in this regime it
  converges in ~3 iterations and the n_iter=1 output is within 4e-5. With
  r1 = u/(K @ 1), the column marginals K^T r1 are already within ~3% of v,
  so c1 ~= 1 and T ~= r1 * K. Then
     sim_op[c,b] = (1/196) sum_m <sim>_w,m
  where <sim>_w,m is the softmax_n(10*sim)-weighted mean of sim over the
  N=4 prompts.

  Two-silu trick: with u = 10*sim, the HW silu spline satisfies
  silu(u - D) = (u-D)*sigmoid(u-D) ~= (u-D)*e^(u-D) to 5e-7 relative for
  u-D <= -8 (verified on HW down to -55). Two ACT passes with shifts D1=18,
  D2=38 give per-element fields P, Q whose n-sums are
     tsP = e^-D1 (A - D1 B),  tsQ = e^-D2 (A - D2 B)
  with A = sum_n u e^u, B = sum_n e^u. Then
     BB = tsP - e^(D2-D1) tsQ          = e^-D1 (D2-D1) B
     AA = tsP - (D1/D2) e^(D2-D1) tsQ  = e^-D1 (D2-D1)/D2 A
     <u>_m = D2 * AA/BB
  This replaces the previous exp+copy ACT pair AND the two DVE K*sim
  products: ACT does 2 passes (same as before), DVE only does sum trees +
  2 scalar-combines + reciprocal + accumulating ratio.
  logits2 = hls*FS + PL where FS accumulates (D2/1960)*rd*AA over m,
  PL = pTn^T @ ipn with pTn pre-scaled by 0.5*exp(ls), hls = 0.5*exp(ls).

Layout: classes on partitions (125/chunk, 8 chunks), free dim = b-pair x
(n, m). Per (j, bp) iteration: 16 PE matmuls -> PSUM tile [125, 4x512-bank];
ACT: 2 wide silu -> P, Q [125,1568] bf16 in (b, n, m) layout; DVE/Pool:
pair-sum trees, scalar combines, reciprocal, fused ratio+accum.
"""

import numpy as np
import ml_dtypes
from contextlib import ExitStack

import concourse.bass as bass
from concourse import bacc
import concourse.tile as tile
import concourse.mybir as mybir
from concourse.bass_utils import run_bass_kernel_spmd

F32 = mybir.dt.float32
BF16 = mybir.dt.bfloat16
F16 = mybir.dt.float16
FP8 = mybir.dt.float8e4
AF = mybir.ActivationFunctionType
OP = mybir.AluOpType
PM = mybir.MatmulPerfMode
FSCALE = 16.0   # host pre-scale of fp8 features; sim arrives x256 in PSUM

M = 196        # image patches
D = 512        # feature dim
N = 4          # prompt ensembles
NCLS = 1000    # classes
BL = 4         # local batch (b=32 / 8 cores)
NCORES = 8
J = 8          # class chunks
CJ = 125       # classes per chunk (partition dim)
KD = 4         # d chunks of 128
D1 = 12.0      # silu shift 1 (u <= 2.5 for normalized features -> u-D1 <= -9.5)
D2 = 50.0      # silu shift 2 (HW silu spline verified exact to -55)

# engine per op kind: 'v' = DVE, 'g' = gpsimd(Pool)
# dma: 'spread' uses ACT+Pool for startup DMAs
DEFAULT_CFG = dict(inbufs=2, dma='spread', stage='full', kbufs=4, tbufs=4,
                   t1P='v', t1Q='g', tsP='v', tsQ='g', bb='v', aa='v',
                   ratio='v', tsdt='f32')


def _kern(ctx: ExitStack, tc: tile.TileContext, cfg, t_out, t_text, t_ptext,
          t_img, t_ipool, t_hls, loop_reps=0):
    nc = tc.nc
    persist = ctx.enter_context(tc.tile_pool(name="persist", bufs=1))

    bias1 = persist.tile([128, 1], F32, tag="bias1", name="bias1")
    bias2 = persist.tile([128, 1], F32, tag="bias2", name="bias2")
    dummy = persist.tile([128, 1], F32, tag="dummy", name="dummy")
    wjunk = persist.tile([128, 392], F32, tag="wjunk", name="wjunk")
    nc.gpsimd.memset(bias1[:], -D1)
    nc.gpsimd.memset(bias2[:], -D2)
    nc.gpsimd.memset(wjunk[:], 1.0)
    # dummy silu at the top: forces the act-table load at t~0 instead of
    # right before the first real silu (the load is ~1.3us + drain)
    nc.scalar.activation(dummy[:], bias1[:], AF.Silu, bias=0.0, scale=1.0)

    in_p = ctx.enter_context(tc.tile_pool(name="inputs", bufs=cfg['inbufs']))
    ps_p = ctx.enter_context(tc.tile_pool(name="mn_ps", bufs=2, space="PSUM"))
    k_p = ctx.enter_context(tc.tile_pool(name="mn_k", bufs=cfg['kbufs']))
    t_p = ctx.enter_context(tc.tile_pool(name="mn_t", bufs=cfg['tbufs']))
    j_p = ctx.enter_context(tc.tile_pool(name="mn_j", bufs=4))
    eng = {'v': nc.vector, 'g': nc.gpsimd}
    TSDT = F32 if cfg['tsdt'] == 'f32' else BF16

    def emit_body():
        # merged tiles: col = k*stride + inner, k = (k2, i) = d-chunk of 128
        # (d = k2*256 + i*128 + p; DoubleRow contracts (p, i) pairs)
        tfT = in_p.tile([128, KD * N * NCLS], FP8, tag="tfT", name="tfT")
        pT = in_p.tile([128, KD * NCLS], BF16, tag="pT", name="pT")
        imT = in_p.tile([128, KD * BL * M], FP8, tag="imT", name="imT")
        ipT = in_p.tile([128, KD * BL], BF16, tag="ipT", name="ipT")
        FS = in_p.tile([CJ, J * BL], F32, tag="FS", name="FS")
        PLf = in_p.tile([CJ, J * BL], F32, tag="PLf", name="PLf")
        hls = in_p.tile([128, 1], F32, tag="hls", name="hls")

        # ---- input DMAs: merged into a few wide transfers, split by j
        # ranges so early chunks land first. hwdge queues: SP/ACT (+Pool
        # swdge). imT gates the first matmul -> first on its own queue.
        c1 = N * CJ          # 500 cols = one j chunk
        e2 = nc.scalar if cfg['dma'] == 'spread' else nc.sync
        e3 = nc.gpsimd if cfg['dma'] == 'spread' else nc.sync
        tf_v = tfT[:].rearrange("p (k c) -> p k c", k=KD)
        tx_v = t_text[:, :].rearrange("p (k c) -> p k c", k=KD)
        # Queue discipline: per-queue DMA sems are cumulative (consumer of
        # DMA #n waits for all #<=n on that queue), so each queue's issue
        # order must match consumption order, and unrelated streams go on
        # different queues. Text on SP in j order (j0 split per k); image
        # on Pool(swdge) split per k; tail-only inputs last on SP. Never
        # issue input DMAs from ACT/DVE queues: pending descgens block
        # those engines' in-order SEQ streams.
        nc.sync.dma_start(tf_v[:, :, 0:c1], tx_v[:, :, 0:c1])
        for kh in range(2):
            e3.dma_start(imT[:, kh * 2 * BL * M:(kh + 1) * 2 * BL * M],
                         t_img[:, kh * 2 * BL * M:(kh + 1) * 2 * BL * M])
        for jj in range(1, J):
            nc.sync.dma_start(tf_v[:, :, jj * c1:(jj + 1) * c1],
                              tx_v[:, :, jj * c1:(jj + 1) * c1])
        nc.sync.dma_start(pT[:], t_ptext[:, :])
        nc.sync.dma_start(ipT[:], t_ipool[:, :])
        nc.sync.dma_start(hls[:], t_hls[:, :])

        stage = cfg['stage']
        if stage == 'dma':
            return
        if True:
            OJ = in_p.tile([CJ, J * BL], F32, tag="OJ", name="OJ")
            PSl = None
            for j in range(J):
                for bp in range(2):
                    # one 4-bank PSUM tile: n on 512-col banks, bm 392 cols
                    PSh = ps_p.tile([CJ, 4 * 512], F32, tag="psh", name="psh")
                    # k2-outer so the first iteration overlaps incoming
                    # per-k DMA chunks; PSUM has_written handles per-bank
                    # accumulation interleave. fp8 DoubleRow: lhsT/rhs are
                    # [p, 2, f] views, contraction = 2x128 rows per chunk.
                    tf4 = tfT[:].rearrange("p (k2 i c) -> p k2 i c",
                                           k2=2, i=2)
                    im4 = imT[:].rearrange("p (k2 i c) -> p k2 i c",
                                           k2=2, i=2)
                    for k2 in range(2):
                        for n in range(4):
                            c0 = j * (N * CJ) + n * CJ
                            nc.tensor.matmul(
                                PSh[:, n * 512:n * 512 + 392],
                                lhsT=tf4[:, k2, :, c0:c0 + CJ],
                                rhs=im4[:, k2, :, bp * 392:(bp + 1) * 392],
                                start=(k2 == 0), stop=(k2 == 1),
                                perf_mode=PM.DoubleRow)
                    if j == 5 and bp == 0:
                        # PL block into spare cols of bank 0, after this
                        # tile's sim matmuls; mid-run so pT/ipT DMAs are
                        # done and the tail stays short
                        PSl = PSh
                        for jj in range(J):
                            for k in range(KD):
                                nc.tensor.matmul(
                                    PSl[:, 392 + jj * BL:392 + (jj + 1) * BL],
                                    lhsT=pT[:, k * NCLS + jj * CJ:
                                            k * NCLS + (jj + 1) * CJ],
                                    rhs=ipT[:, k * BL:(k + 1) * BL],
                                    start=(k == 0), stop=(k == KD - 1))
                        nc.scalar.activation(PLf[:], PSl[:, 392:392 + J * BL],
                                             AF.Copy, bias=0.0, scale=1.0)
                    if stage == 'mm':
                        continue
                    # (n, b, m) view of the sim part of PSUM
                    psv = PSh[:].rearrange("p (q s) -> p q s", q=4)[:, :, 0:392] \
                        .rearrange("p q (b m) -> p q b m", b=2, m=M)
                    # P/Q written b-major: col b*784 + n*196 + m
                    Pw = k_p.tile([CJ, 2 * N * M], BF16, tag="P", name="P")
                    Qw = k_p.tile([CJ, 2 * N * M], BF16, tag="Q", name="Q")
                    pv = Pw[:].rearrange("p (b q m) -> p q b m", b=2, q=N)
                    qv = Qw[:].rearrange("p (b q m) -> p q b m", b=2, q=N)
                    sc = 10.0 / (FSCALE * FSCALE)
                    nc.scalar.activation(pv, psv, AF.Silu,
                                         bias=bias1[0:CJ, :], scale=sc)
                    nc.scalar.activation(qv, psv, AF.Silu,
                                         bias=bias2[0:CJ, :], scale=sc)

                    if stage == 'act':
                        continue
                    # --- pair-sum trees + combines; (b, n, m) layout ---
                    t1P = t_p.tile([CJ, 2 * 2 * M], BF16, tag="t1P", name="t1P")
                    t1Q = t_p.tile([CJ, 2 * 2 * M], BF16, tag="t1Q", name="t1Q")
                    tsP = t_p.tile([CJ, 2 * M], TSDT, tag="tsP", name="tsP")
                    tsQ = t_p.tile([CJ, 2 * M], TSDT, tag="tsQ", name="tsQ")
                    BB = t_p.tile([CJ, 2 * M], F32, tag="BB", name="BB")
                    AA = t_p.tile([CJ, 2 * M], TSDT, tag="AA", name="AA")
                    rd = t_p.tile([CJ, 2 * M], F32, tag="rd", name="rd")

                    def halves(tile_, w):
                        v = tile_[:].rearrange("p (b h x) -> p b h x", b=2, h=2)
                        return v[:, :, 0, :], v[:, :, 1, :]

                    e_t1Q = cfg['t1Q']
                    e_tsQ = cfg['tsQ']
                    a0, a1 = halves(Pw, 392)
                    eng[cfg['t1P']].tensor_add(
                        t1P[:].rearrange("p (b x) -> p b x", b=2), a0, a1)
                    b0, b1 = halves(Qw, 392)
                    eng[e_t1Q].tensor_add(
                        t1Q[:].rearrange("p (b x) -> p b x", b=2), b0, b1)
                    c0_, c1_ = halves(t1P, 196)
                    eng[cfg['tsP']].tensor_add(
                        tsP[:].rearrange("p (b x) -> p b x", b=2), c0_, c1_)
                    d0, d1 = halves(t1Q, 196)
                    eng[e_tsQ].tensor_add(
                        tsQ[:].rearrange("p (b x) -> p b x", b=2), d0, d1)
                    if stage == 'trees':
                        continue
                    E21 = float(np.exp(D2 - D1))
                    eng[cfg['bb']].scalar_tensor_tensor(
                        out=BB[:], in0=tsQ[:], scalar=-E21, in1=tsP[:],
                        op0=OP.mult, op1=OP.add)
                    eng[cfg['aa']].scalar_tensor_tensor(
                        out=AA[:], in0=tsQ[:], scalar=-(D1 / D2) * E21,
                        in1=tsP[:], op0=OP.mult, op1=OP.add)
                    nc.vector.reciprocal_approx_fast(out=rd[:], in_=BB[:])
                    for b in range(2):
                        bm = slice(b * M, (b + 1) * M)
                        junk = j_p.tile([CJ, M], F32, tag="jk", name="jk")
                        col = j * BL + bp * 2 + b
                        eng[cfg['ratio']].scalar_tensor_tensor(
                            out=junk[:], in0=rd[:, bm], scalar=D2 / 1960.0,
                            in1=AA[:, bm], op0=OP.mult, op1=OP.mult,
                            accum_out=FS[:, col:col + 1])

            if stage == 'full':
                nc.vector.scalar_tensor_tensor(
                    out=OJ[:], in0=FS[:], scalar=hls[0:CJ, :], in1=PLf[:],
                    op0=OP.mult, op1=OP.add)
                nc.sync.dma_start(t_out[:, :], OJ[:])

    if loop_reps:
        # two bodies per hw-loop iteration: with inbufs=2 the input tiles
        # alternate buffers, so body B's input DMAs overlap body A's
        # compute (and A's DMAs overlap B's compute across the loop edge)
        assert loop_reps % 2 == 0, "loop_reps must be even"
        with tc.For_i(0, loop_reps // 2, 1):
            emit_body()
            emit_body()
    else:
        emit_body()


_CACHE = {}


def _get_compiled(loop_reps=0, cfg=None):
    cfg = dict(DEFAULT_CFG, **(cfg or {}))
    key = (loop_reps, tuple(sorted(cfg.items())))
    if key in _CACHE:
        return _CACHE[key]
    nc = bacc.Bacc("TRN2", target_bir_lowering=False, debug=False,
                   enable_asserts=False, num_devices=NCORES)
    t_text = nc.dram_tensor("text_bf16", [128, KD * N * NCLS], FP8,
                            kind="ExternalInput").ap()
    t_ptext = nc.dram_tensor("ptext_bf16", [128, KD * NCLS], BF16,
                             kind="ExternalInput").ap()
    t_img = nc.dram_tensor("img", [128, KD * BL * M], FP8,
                           kind="ExternalInput").ap()
    t_ipool = nc.dram_tensor("imgpool", [128, KD * BL], BF16,
                             kind="ExternalInput").ap()
    t_hls = nc.dram_tensor("half_ls", [128, 1], F32, kind="ExternalInput").ap()
    t_out = nc.dram_tensor("out", [CJ, J * BL], F32, kind="ExternalOutput").ap()
    with tile.TileContext(nc) as tc:
        with ExitStack() as ctx:
            _kern(ctx, tc, cfg, t_out, t_text, t_ptext, t_img, t_ipool, t_hls,
                  loop_reps=loop_reps)
    nc.compile()
    _CACHE[key] = nc
    return nc


def _host_prep(image_features, image_feature_pool, text_features, logit_scale):
    """Normalize + transpose + cast on host; returns per-core input maps."""
    bf16 = ml_dtypes.bfloat16
    imf = np.asarray(image_features, np.float32)          # [196, 32, 512]
    ipool = np.asarray(image_feature_pool, np.float32)    # [32, 512]
    text = np.asarray(text_features, np.float32)          # [4000, 512]
    ls = float(np.asarray(logit_scale, np.float32).reshape(()))

    tf = text.reshape(N, NCLS, D)
    tpool = tf.mean(axis=0)
    tpool_n = tpool / np.linalg.norm(tpool, axis=1, keepdims=True)
    tfn = tf / np.linalg.norm(tf, axis=2, keepdims=True)

    hls_v = 0.5 * np.exp(ls)

    fp8 = ml_dtypes.float8_e4m3fn

    def kfold(x):
        """[512, C] -> [128, 4*C] with col = k*C + c (k = row-chunk of 128)."""
        c = x.shape[1]
        return np.ascontiguousarray(
            x.reshape(KD, 128, c).transpose(1, 0, 2).reshape(128, KD * c))

    # text cols: j*500 + n*125 + cc  (j-major for early-chunk DMA)
    tfn_r = tfn.reshape(N, J, CJ, D).transpose(3, 1, 0, 2).reshape(D, N * NCLS)
    text_fp8 = kfold(tfn_r * 16.0).astype(fp8)
    ptext_bf16 = kfold(tpool_n.T * hls_v).astype(bf16)   # [128, 4*1000]

    imn = imf / np.linalg.norm(imf, axis=2, keepdims=True)  # [196, 32, 512]
    ipn = ipool / np.linalg.norm(ipool, axis=1, keepdims=True)
    hls = np.full((128, 1), hls_v, dtype=np.float32)

    in_maps = []
    for core in range(NCORES):
        sl = slice(core * BL, (core + 1) * BL)
        # img: [128, k*784 + b*196 + m]
        img_c = kfold(imn[:, sl, :].transpose(2, 1, 0).reshape(D, BL * M)
                      * 16.0).astype(fp8)
        ip_c = kfold(ipn[sl].T).astype(bf16)   # [128, 16]
        in_maps.append({
            "text_bf16": text_fp8,
            "ptext_bf16": ptext_bf16,
            "img": img_c,
            "imgpool": ip_c,
            "half_ls": hls,
        })
    return in_maps


def kernel(image_features, image_feature_pool, text_features, logit_scale):
    nc = _get_compiled()
    in_maps = _host_prep(image_features, image_feature_pool, text_features,
                         logit_scale)
    res = run_bass_kernel_spmd(nc, in_maps, core_ids=list(range(NCORES)))
    outs = [unscramble_out(np.asarray(res.results[i]["out"], np.float32))
            for i in range(NCORES)]
    return np.concatenate(outs, axis=0)


def unscramble_out(o):
    """[125, J*BL] device tile -> [BL, NCLS] logits block for one core."""
    return o.reshape(CJ, J, BL).transpose(2, 1, 0).reshape(BL, NCLS)
